# revision 20
# baseline (speedup 1.0000x reference)
"""PixelContrastLoss forward on 8 Trainium2 cores.

Contract: kernel(feats [8192,1,128] f32, labels [8192] i32) -> scalar f32 loss,
matching:
    logits = (F @ F.T) / T;  l = logits - rowmax
    NL_i = sum_{j: label_j != label_i} exp(l_ij)
    loss = -(T/BT) * mean_i [ sum_{j in pos, j!=i} (l_ij - log(exp(l_ij)+NL_i))
                              / (cnt_i - 1) ]

Math used by the kernel:
  * l - log(exp l + NL) is invariant to the rowmax shift, so we shift by the
    constant 10 (= diag logit of L2-normalized rows / T) instead of computing
    the row max.
  * Rows/columns are sorted by label on the host (loss is invariant under
    permutation), so each row's positive/negative column sets are the two
    compile-time ranges [0,cnt0) and [cnt0,N).
  * sum_pos l = 10*sum_pos r - 10*cntA comes from plain row-sums of the raw
    logits r (DVE reduce of PSUM), no elementwise pass.
  * diag correction: l_ii ~= 0, so subtracting the self-pair contributes
    +log1p(NL_i) to the row sum.

Each core runs the same program (SPMD) on 9 row-blocks of 128 rows; the first
k0 = ceil(cnt0/1024) blocks hold label-0 rows, the rest label-1 rows. The
9*128*8 = 9216 row slots cover the 8192 real rows; padded slots are duplicate
rows with weight 0.
"""

import math
from contextlib import ExitStack

import numpy as np
import ml_dtypes

import concourse.bass as bass
import concourse.tile as tile
from concourse import bacc, mybir
from concourse.bass_utils import run_bass_kernel_spmd

N = 8192
D = 128
NCORES = 8
RB = 9                     # row-blocks per core
PBLK = 128                 # rows per block
SLOTS = RB * PBLK          # row slots per core
TEMPERATURE = 0.1
BASE_TEMPERATURE = 0.07
PS_W = 2048                # psum tile width (4 banks), f32
MM_W = 512                 # max matmul free dim (one psum bank of f32)
FTB_W = 2048               # ftb column-tile width


def _cuts(c0, c1, step, extra=(), lead=None):
    """Split [c0,c1) at multiples of `step` relative to c0 (or after an
    optional smaller `lead` first piece), plus extra absolute cut points."""
    cuts = {c0, c1}
    a = c0 + lead if lead else c0
    cuts |= set(range(a, c1, step))
    cuts |= {c for c in extra if c0 < c < c1}
    cuts = sorted(cuts)
    return [(x, y - x) for x, y in zip(cuts, cuts[1:])]


def _build(cnt0):
    """Emit the SPMD program for one core, parametrized by the label split."""
    cnt1 = N - cnt0
    k0 = math.ceil(cnt0 / 1024)
    f32 = mybir.dt.float32
    bf16 = mybir.dt.bfloat16
    Exp = mybir.ActivationFunctionType.Exp
    Ln = mybir.ActivationFunctionType.Ln
    X = mybir.AxisListType.X
    ADD = mybir.AluOpType.add

    posmax = max(cnt0, cnt1, 1)

    nc = bacc.Bacc("TRN2", target_bir_lowering=False, debug=False,
                   enable_asserts=False, num_devices=NCORES)

    ftb_d = nc.dram_tensor("ftb", [D, N], bf16, kind="ExternalInput").ap()
    ftr_d = nc.dram_tensor("ftr", [D, SLOTS], bf16, kind="ExternalInput").ap()
    w_d = nc.dram_tensor("w", [PBLK, RB], f32, kind="ExternalInput").ap()
    out_d = nc.dram_tensor("out", [PBLK, RB], f32, kind="ExternalOutput").ap()

    with tile.TileContext(nc) as tc, ExitStack() as ctx:
        io = ctx.enter_context(tc.tile_pool(name="io", bufs=1))
        ftbp = ctx.enter_context(tc.tile_pool(name="ftbp", bufs=N // FTB_W))
        psp = ctx.enter_context(tc.tile_pool(name="psp", bufs=2, space="PSUM"))
        scr = ctx.enter_context(tc.tile_pool(name="scr", bufs=2))
        rowp = ctx.enter_context(tc.tile_pool(name="rowp", bufs=2))

        # --- inputs resident in SBUF ---
        # ftb segments anchored at rb0's first-needed range (its negative
        # range), with a small 512-wide lead segment so the first matmul can
        # start as soon as ~160KB have landed. Loads spread over 3 queues.
        rb0neg = (cnt0, N) if k0 > 0 else (0, cnt0)
        rb0pos = (0, cnt0) if k0 > 0 else (cnt0, N)
        segs = []
        for (a, b), lead in ((rb0neg, 512), (rb0pos, None)):
            if b > a:
                segs.extend(_cuts(a, b, FTB_W, lead=lead if b - a > 512 else None))

        # first the lead segment + rb0's lhsT slice, then everything else
        ftr0 = io.tile([D, PBLK], bf16)
        nc.sync.dma_start(out=ftr0[:], in_=ftr_d[:, 0:PBLK])
        engines = [nc.gpsimd, nc.sync, nc.scalar]
        ftbt = []
        for j, (g0, w) in enumerate(segs):
            fb = ftbp.tile([D, w], bf16, name=f"ftb{j}", tag=f"ftb{j}")
            engines[j % len(engines)].dma_start(out=fb[:], in_=ftb_d[:, g0:g0 + w])
            ftbt.append((g0, w, fb))
        seg_bounds = [g0 for g0, _, _ in ftbt]
        ftr1 = io.tile([D, SLOTS - PBLK], bf16)
        nc.gpsimd.dma_start(out=ftr1[:], in_=ftr_d[:, PBLK:])
        wt = io.tile([PBLK, RB], f32)
        nc.sync.dma_start(out=wt[:], in_=w_d[:])
        b10 = io.tile([PBLK, 1], f32)
        nc.vector.memset(b10[:], -10.0)
        b10p = io.tile([PBLK, 1], f32)
        nc.vector.memset(b10p[:], 10.0)
        cout = io.tile([PBLK, RB], f32)

        def mm_range(lhsT, c0, c1, act_fn, lead=None):
            """Matmul columns [c0,c1) into psum tiles; per psum tile call
            act_fn(ps, width, tile_index)."""
            for ti, (t0, tw) in enumerate(_cuts(c0, c1, PS_W, lead=lead)):
                ps = psp.tile([PBLK, PS_W], f32, name="ps", tag="ps")
                for (p0, pw) in _cuts(t0, t0 + tw, MM_W, extra=seg_bounds):
                    g0, gw, fb = next(
                        s for s in ftbt if s[0] <= p0 and p0 + pw <= s[0] + s[1])
                    nc.tensor.matmul(
                        ps[:, p0 - t0:p0 - t0 + pw],
                        lhsT,
                        fb[:, p0 - g0:p0 - g0 + pw],
                        start=True, stop=True)
                act_fn(ps, tw, ti)

        for rb in range(RB):
            is0 = rb < k0
            pos = (0, cnt0) if is0 else (cnt0, N)
            neg = (cnt0, N) if is0 else (0, cnt0)
            lhsT = ftr0[:] if rb == 0 else ftr1[:, (rb - 1) * PBLK:rb * PBLK]

            negw = neg[1] - neg[0]
            posw = pos[1] - pos[0]
            lead = 512 if rb == 0 and negw > 512 else None
            nnegt = max(1, len(_cuts(neg[0], neg[1], PS_W, lead=lead)) if negw else 1)

            # --- negative columns: exp(10r-10), row-accumulate -> NL ---
            at = rowp.tile([PBLK, nnegt], f32, name="at", tag="at")

            def neg_act(ps, tw, ti, at=at):
                ts = scr.tile([PBLK, PS_W], f32, name="ts", tag="ts")
                nc.scalar.activation(ts[:, :tw], ps[:, :tw], Exp,
                                     bias=b10[:], scale=10.0,
                                     accum_out=at[:, ti:ti + 1])

            if negw:
                mm_range(lhsT, neg[0], neg[1], neg_act, lead=lead)

            nl = rowp.tile([PBLK, 1], f32, name="nl", tag="nl")
            if negw:
                nc.vector.tensor_reduce(nl[:], at[:], X, ADD)
            else:
                nc.vector.memset(nl[:], 0.0)
            # extra ln-pass column v with NL*v+1 = 1/(1+NL), so its ln term
            # contributes -log1p(NL): the accumulator then yields -S directly.
            nl1 = rowp.tile([PBLK, 1], f32, name="nl1", tag="nl1")
            nc.vector.tensor_scalar_add(nl1[:], nl[:], 1.0)
            rv = rowp.tile([PBLK, 1], f32, name="rv", tag="rv")
            nc.vector.reciprocal(rv[:], nl1[:])

            # --- positive columns: e = exp(-l) = exp(-10r + 10); NL enters the
            # ln pass as a per-row scale, so this pass has no NL dependency ---
            tp = scr.tile([PBLK, posmax + 1], f32, name="tp", tag="tp")

            def pos_act(ps, tw, ti, tp=tp):
                off = ti * PS_W
                nc.scalar.activation(tp[:, off:off + tw], ps[:, :tw], Exp,
                                     bias=b10p[:], scale=-10.0)

            mm_range(lhsT, pos[0], pos[1], pos_act)

            # --- sum_pos [l - ln(exp l + NL)] = -sum_pos ln(1 + NL*e) ---
            nc.vector.tensor_scalar_mul(tp[:, posw:posw + 1], rv[:], -1.0)
            tz = scr.tile([PBLK, posmax + 1], f32, name="tz", tag="tz")
            sz = rowp.tile([PBLK, 1], f32, name="sz", tag="sz")
            nc.scalar.activation(tz[:, :posw + 1], tp[:, :posw + 1], Ln,
                                 bias=1.0, scale=nl[:], accum_out=sz[:])

            # --- row combine: sz = -(S); host weights carry the sign flip ---
            nc.vector.tensor_mul(cout[:, rb:rb + 1], sz[:], wt[:, rb:rb + 1])

        nc.sync.dma_start(out=out_d[:], in_=cout[:])

    _pin_act_table(nc)
    nc.compile()
    return nc


def _pin_act_table(nc, set_name="natural_log_exp_and_others"):
    """The default table chooser alternates exp-only/ln-only sets, paying a
    ~1.3us ACT_TABLE_LOAD on every Exp<->Ln transition. All our activation
    funcs live in one named set; offer the pass only that set (with a
    coverage check so a future func change falls back safely)."""
    from concourse.hw_specs import get_activation_tables
    from concourse import _compat  # noqa: F401
    import concourse.bacc as bacc_mod

    used = {
        inst.func
        for b in nc.main_func.blocks
        for inst in b.instructions
        if isinstance(inst, mybir.InstActivation)
    }
    tables = get_activation_tables(nc.m.arch)
    pinned = tables.get(set_name)
    if pinned is None or not used.issubset(pinned):
        return  # fall back to default behavior

    # act_func_set_id is the INDEX into act_info.json's act_func_sets, so the
    # list must keep its length/order; empty the other sets so the chooser
    # can only pick ours.
    masked = [(n, f if n == set_name else set()) for n, f in tables.items()]

    def patched():
        bacc_mod._bass_rust.insert_act_table_loads(nc, masked)

    nc.insert_act_table_loads = patched


def _prepare(feats, labels):
    """Host-side sharding prep: sort by label, build per-core inputs."""
    F = np.ascontiguousarray(np.asarray(feats, dtype=np.float32).reshape(N, D))
    lab = np.asarray(labels).reshape(N)
    perm = np.argsort(lab, kind="stable")
    Fs = F[perm]
    ys = lab[perm]
    cnt0 = int(np.searchsorted(ys, 1))
    cnt1 = N - cnt0
    k0 = math.ceil(cnt0 / 1024)

    ftb = np.ascontiguousarray(Fs.T.astype(ml_dtypes.bfloat16))  # [D, N]

    # global slot lists (indices into sorted order) + weights
    n0slots = NCORES * k0 * PBLK
    n1slots = NCORES * (RB - k0) * PBLK
    idx0 = np.full(n0slots, max(cnt0 - 1, 0), dtype=np.int64)
    idx0[:cnt0] = np.arange(cnt0)
    w0 = np.zeros(n0slots, dtype=np.float32)
    if cnt0 > 1:
        w0[:cnt0] = -1.0 / (cnt0 - 1)  # negated: device emits -S per row
    idx1 = np.full(n1slots, N - 1, dtype=np.int64)
    idx1[:cnt1] = cnt0 + np.arange(cnt1)
    w1 = np.zeros(n1slots, dtype=np.float32)
    if cnt1 > 1:
        w1[:cnt1] = -1.0 / (cnt1 - 1)

    idx0 = idx0.reshape(NCORES, k0 * PBLK)
    w0 = w0.reshape(NCORES, k0 * PBLK)
    idx1 = idx1.reshape(NCORES, (RB - k0) * PBLK)
    w1 = w1.reshape(NCORES, (RB - k0) * PBLK)

    in_maps = []
    for c in range(NCORES):
        rows = np.concatenate([idx0[c], idx1[c]])
        ftr = np.ascontiguousarray(Fs[rows].T.astype(ml_dtypes.bfloat16))
        wc = np.concatenate([w0[c], w1[c]]).reshape(RB, PBLK).T.copy()
        in_maps.append({"ftb": ftb, "ftr": ftr, "w": wc})
    return cnt0, in_maps


def _assemble(results):
    total = 0.0
    for r in results:
        total += float(r["out"].astype(np.float64).sum())
    return np.float32(-(TEMPERATURE / BASE_TEMPERATURE) * total / N)


_CACHE = {}


def kernel(feats, labels):
    cnt0, in_maps = _prepare(feats, labels)
    nc = _CACHE.get(cnt0)
    if nc is None:
        nc = _CACHE[cnt0] = _build(cnt0)
    res = run_bass_kernel_spmd(nc, in_maps, list(range(NCORES))).results
    return _assemble(res)


# revision 21
# speedup vs baseline: 1.1577x; 1.1577x over previous
"""PixelContrastLoss forward on 8 Trainium2 cores.

Contract: kernel(feats [8192,1,128] f32, labels [8192] i32) -> scalar f32 loss,
matching:
    logits = (F @ F.T) / T;  l = logits - rowmax
    NL_i = sum_{j: label_j != label_i} exp(l_ij)
    loss = -(T/BT) * mean_i [ sum_{j in pos, j!=i} (l_ij - log(exp(l_ij)+NL_i))
                              / (cnt_i - 1) ]

Math used by the kernel:
  * l - log(exp l + NL) is invariant to the rowmax shift, so we shift by the
    constant 10 (= diag logit of L2-normalized rows / T) instead of computing
    the row max.
  * Rows/columns are sorted by label on the host (loss is invariant under
    permutation), so each row's positive/negative column sets are the two
    compile-time ranges [0,cnt0) and [cnt0,N).
  * sum_pos l = 10*sum_pos r - 10*cntA comes from plain row-sums of the raw
    logits r (DVE reduce of PSUM), no elementwise pass.
  * diag correction: l_ii ~= 0, so subtracting the self-pair contributes
    +log1p(NL_i) to the row sum.

Each core runs the same program (SPMD) on 9 row-blocks of 128 rows; the first
k0 = ceil(cnt0/1024) blocks hold label-0 rows, the rest label-1 rows. The
9*128*8 = 9216 row slots cover the 8192 real rows; padded slots are duplicate
rows with weight 0.
"""

import math
from contextlib import ExitStack

import numpy as np
import ml_dtypes

import concourse.bass as bass
import concourse.tile as tile
from concourse import bacc, mybir
from concourse.bass_utils import run_bass_kernel_spmd

N = 8192
D = 128
NCORES = 8
RB = 9                     # row-blocks per core
PBLK = 128                 # rows per block
SLOTS = RB * PBLK          # row slots per core
TEMPERATURE = 0.1
BASE_TEMPERATURE = 0.07
PS_W = 2048                # psum tile width (4 banks), f32
MM_W = 512                 # max matmul free dim (one psum bank of f32)
FTB_W = 2048               # ftb column-tile width


def _cuts(c0, c1, step, extra=(), lead=None):
    """Split [c0,c1) at multiples of `step` relative to c0 (or after an
    optional smaller `lead` first piece), plus extra absolute cut points."""
    cuts = {c0, c1}
    a = c0 + lead if lead else c0
    cuts |= set(range(a, c1, step))
    cuts |= {c for c in extra if c0 < c < c1}
    cuts = sorted(cuts)
    return [(x, y - x) for x, y in zip(cuts, cuts[1:])]


def _build(cnt0):
    """Emit the SPMD program for one core, parametrized by the label split."""
    cnt1 = N - cnt0
    k0 = math.ceil(cnt0 / 1024)
    f32 = mybir.dt.float32
    bf16 = mybir.dt.bfloat16
    Exp = mybir.ActivationFunctionType.Exp
    Ln = mybir.ActivationFunctionType.Ln
    X = mybir.AxisListType.X
    ADD = mybir.AluOpType.add

    posmax = max(cnt0, cnt1, 1)

    nc = bacc.Bacc("TRN2", target_bir_lowering=False, debug=False,
                   enable_asserts=False, num_devices=NCORES)

    ftb_d = nc.dram_tensor("ftb", [D, N], bf16, kind="ExternalInput").ap()
    ftr_d = nc.dram_tensor("ftr", [D, SLOTS], bf16, kind="ExternalInput").ap()
    w_d = nc.dram_tensor("w", [PBLK, RB], f32, kind="ExternalInput").ap()
    out_d = nc.dram_tensor("out", [PBLK, RB], f32, kind="ExternalOutput").ap()

    with tile.TileContext(nc) as tc, ExitStack() as ctx:
        io = ctx.enter_context(tc.tile_pool(name="io", bufs=1))
        ftbp = ctx.enter_context(tc.tile_pool(name="ftbp", bufs=N // FTB_W))
        psp = ctx.enter_context(tc.tile_pool(name="psp", bufs=2, space="PSUM"))
        scr = ctx.enter_context(tc.tile_pool(name="scr", bufs=2))
        rowp = ctx.enter_context(tc.tile_pool(name="rowp", bufs=2))

        # --- inputs resident in SBUF ---
        # ftb segments anchored at rb0's first-needed range (its negative
        # range), with a small 512-wide lead segment so the first matmul can
        # start as soon as ~160KB have landed. Loads spread over 3 queues.
        rb0neg = (cnt0, N) if k0 > 0 else (0, cnt0)
        rb0pos = (0, cnt0) if k0 > 0 else (cnt0, N)
        segs = []
        for (a, b), lead in ((rb0neg, 512), (rb0pos, None)):
            if b > a:
                segs.extend(_cuts(a, b, FTB_W, lead=lead if b - a > 512 else None))

        # first the lead segment + rb0's lhsT slice, then everything else
        ftr0 = io.tile([D, PBLK], bf16)
        nc.sync.dma_start(out=ftr0[:], in_=ftr_d[:, 0:PBLK])
        engines = [nc.gpsimd, nc.sync, nc.scalar]
        ftbt = []
        for j, (g0, w) in enumerate(segs):
            fb = ftbp.tile([D, w], bf16, name=f"ftb{j}", tag=f"ftb{j}")
            engines[j % len(engines)].dma_start(out=fb[:], in_=ftb_d[:, g0:g0 + w])
            ftbt.append((g0, w, fb))
        seg_bounds = [g0 for g0, _, _ in ftbt]
        ftr1 = io.tile([D, SLOTS - PBLK], bf16)
        nc.gpsimd.dma_start(out=ftr1[:], in_=ftr_d[:, PBLK:])
        wt = io.tile([PBLK, RB], f32)
        nc.sync.dma_start(out=wt[:], in_=w_d[:])
        b10 = io.tile([PBLK, 1], f32)
        nc.vector.memset(b10[:], -10.0)
        b10p = io.tile([PBLK, 1], f32)
        nc.vector.memset(b10p[:], 10.0)
        cout = io.tile([PBLK, RB], f32)

        def mm_range(lhsT, c0, c1, act_fn, lead=None):
            """Matmul columns [c0,c1) into psum tiles; per psum tile call
            act_fn(ps, width, tile_index)."""
            for ti, (t0, tw) in enumerate(_cuts(c0, c1, PS_W, lead=lead)):
                ps = psp.tile([PBLK, PS_W], f32, name="ps", tag="ps")
                for (p0, pw) in _cuts(t0, t0 + tw, MM_W, extra=seg_bounds):
                    g0, gw, fb = next(
                        s for s in ftbt if s[0] <= p0 and p0 + pw <= s[0] + s[1])
                    nc.tensor.matmul(
                        ps[:, p0 - t0:p0 - t0 + pw],
                        lhsT,
                        fb[:, p0 - g0:p0 - g0 + pw],
                        start=True, stop=True)
                act_fn(ps, tw, ti)

        for rb in range(RB):
            is0 = rb < k0
            pos = (0, cnt0) if is0 else (cnt0, N)
            neg = (cnt0, N) if is0 else (0, cnt0)
            lhsT = ftr0[:] if rb == 0 else ftr1[:, (rb - 1) * PBLK:rb * PBLK]

            negw = neg[1] - neg[0]
            posw = pos[1] - pos[0]
            lead = 512 if rb == 0 and negw > 512 else None
            nnegt = max(1, len(_cuts(neg[0], neg[1], PS_W, lead=lead)) if negw else 1)

            # --- negative columns: exp(10r-10), row-accumulate -> NL ---
            at = rowp.tile([PBLK, nnegt], f32, name="at", tag="at")

            def neg_act(ps, tw, ti, at=at):
                ts = scr.tile([PBLK, PS_W], f32, name="ts", tag="ts")
                nc.scalar.activation(ts[:, :tw], ps[:, :tw], Exp,
                                     bias=b10[:], scale=10.0,
                                     accum_out=at[:, ti:ti + 1])

            if negw:
                mm_range(lhsT, neg[0], neg[1], neg_act, lead=lead)

            nl = rowp.tile([PBLK, 1], f32, name="nl", tag="nl")
            if negw:
                nc.vector.tensor_reduce(nl[:], at[:], X, ADD)
            else:
                nc.vector.memset(nl[:], 0.0)
            l1p = rowp.tile([PBLK, 1], f32, name="l1p", tag="l1p")
            nc.scalar.activation(l1p[:], nl[:], Ln, bias=1.0, scale=1.0)

            # --- positive columns: e = exp(-l) = exp(-10r + 10); NL enters the
            # ln pass as a per-row scale, so this pass has no NL dependency ---
            tp = scr.tile([PBLK, posmax], f32, name="tp", tag="tp")

            def pos_act(ps, tw, ti, tp=tp):
                off = ti * PS_W
                nc.scalar.activation(tp[:, off:off + tw], ps[:, :tw], Exp,
                                     bias=b10p[:], scale=-10.0)

            mm_range(lhsT, pos[0], pos[1], pos_act)

            # --- sum_pos [l - ln(exp l + NL)] = -sum_pos ln(1 + NL*e) ---
            tz = scr.tile([PBLK, posmax], f32, name="tz", tag="tz")
            sz = rowp.tile([PBLK, 1], f32, name="sz", tag="sz")
            nc.scalar.activation(tz[:, :posw], tp[:, :posw], Ln,
                                 bias=1.0, scale=nl[:], accum_out=sz[:])

            # --- row combine: S = log1p(NL) - sz; out = S*w (w negated) ---
            s2 = rowp.tile([PBLK, 1], f32, name="s2", tag="s2")
            nc.vector.tensor_sub(s2[:], sz[:], l1p[:])
            nc.vector.tensor_mul(cout[:, rb:rb + 1], s2[:], wt[:, rb:rb + 1])

        nc.sync.dma_start(out=out_d[:], in_=cout[:])

    _pin_act_table(nc)
    nc.compile()
    return nc


def _pin_act_table(nc, set_name="natural_log_exp_and_others"):
    """The default table chooser alternates exp-only/ln-only sets, paying a
    ~1.3us ACT_TABLE_LOAD on every Exp<->Ln transition. All our activation
    funcs live in one named set; offer the pass only that set (with a
    coverage check so a future func change falls back safely)."""
    from concourse.hw_specs import get_activation_tables
    from concourse import _compat  # noqa: F401
    import concourse.bacc as bacc_mod

    used = {
        inst.func
        for b in nc.main_func.blocks
        for inst in b.instructions
        if isinstance(inst, mybir.InstActivation)
    }
    tables = get_activation_tables(nc.m.arch)
    pinned = tables.get(set_name)
    if pinned is None or not used.issubset(pinned):
        return  # fall back to default behavior

    # act_func_set_id is the INDEX into act_info.json's act_func_sets, so the
    # list must keep its length/order; empty the other sets so the chooser
    # can only pick ours.
    masked = [(n, f if n == set_name else set()) for n, f in tables.items()]

    def patched():
        bacc_mod._bass_rust.insert_act_table_loads(nc, masked)

    nc.insert_act_table_loads = patched


def _prepare(feats, labels):
    """Host-side sharding prep: sort by label, build per-core inputs."""
    F = np.ascontiguousarray(np.asarray(feats, dtype=np.float32).reshape(N, D))
    lab = np.asarray(labels).reshape(N)
    perm = np.argsort(lab, kind="stable")
    Fs = F[perm]
    ys = lab[perm]
    cnt0 = int(np.searchsorted(ys, 1))
    cnt1 = N - cnt0
    k0 = math.ceil(cnt0 / 1024)

    ftb = np.ascontiguousarray(Fs.T.astype(ml_dtypes.bfloat16))  # [D, N]

    # global slot lists (indices into sorted order) + weights
    n0slots = NCORES * k0 * PBLK
    n1slots = NCORES * (RB - k0) * PBLK
    idx0 = np.full(n0slots, max(cnt0 - 1, 0), dtype=np.int64)
    idx0[:cnt0] = np.arange(cnt0)
    w0 = np.zeros(n0slots, dtype=np.float32)
    if cnt0 > 1:
        w0[:cnt0] = -1.0 / (cnt0 - 1)  # negated: device emits -S per row
    idx1 = np.full(n1slots, N - 1, dtype=np.int64)
    idx1[:cnt1] = cnt0 + np.arange(cnt1)
    w1 = np.zeros(n1slots, dtype=np.float32)
    if cnt1 > 1:
        w1[:cnt1] = -1.0 / (cnt1 - 1)

    idx0 = idx0.reshape(NCORES, k0 * PBLK)
    w0 = w0.reshape(NCORES, k0 * PBLK)
    idx1 = idx1.reshape(NCORES, (RB - k0) * PBLK)
    w1 = w1.reshape(NCORES, (RB - k0) * PBLK)

    in_maps = []
    for c in range(NCORES):
        rows = np.concatenate([idx0[c], idx1[c]])
        ftr = np.ascontiguousarray(Fs[rows].T.astype(ml_dtypes.bfloat16))
        wc = np.concatenate([w0[c], w1[c]]).reshape(RB, PBLK).T.copy()
        in_maps.append({"ftb": ftb, "ftr": ftr, "w": wc})
    return cnt0, in_maps


def _assemble(results):
    total = 0.0
    for r in results:
        total += float(r["out"].astype(np.float64).sum())
    return np.float32(-(TEMPERATURE / BASE_TEMPERATURE) * total / N)


_CACHE = {}


def kernel(feats, labels):
    cnt0, in_maps = _prepare(feats, labels)
    nc = _CACHE.get(cnt0)
    if nc is None:
        nc = _CACHE[cnt0] = _build(cnt0)
    res = run_bass_kernel_spmd(nc, in_maps, list(range(NCORES))).results
    return _assemble(res)


# revision 24
# speedup vs baseline: 1.1702x; 1.0107x over previous
"""PixelContrastLoss forward on 8 Trainium2 cores.

Contract: kernel(feats [8192,1,128] f32, labels [8192] i32) -> scalar f32 loss,
matching:
    logits = (F @ F.T) / T;  l = logits - rowmax
    NL_i = sum_{j: label_j != label_i} exp(l_ij)
    loss = -(T/BT) * mean_i [ sum_{j in pos, j!=i} (l_ij - log(exp(l_ij)+NL_i))
                              / (cnt_i - 1) ]

Math used by the kernel:
  * l - log(exp l + NL) is invariant to the rowmax shift, so we shift by the
    constant 10 (= diag logit of L2-normalized rows / T) instead of computing
    the row max.
  * Rows/columns are sorted by label on the host (loss is invariant under
    permutation), so each row's positive/negative column sets are the two
    compile-time ranges [0,cnt0) and [cnt0,N).
  * sum_pos l = 10*sum_pos r - 10*cntA comes from plain row-sums of the raw
    logits r (DVE reduce of PSUM), no elementwise pass.
  * diag correction: l_ii ~= 0, so subtracting the self-pair contributes
    +log1p(NL_i) to the row sum.

Each core runs the same program (SPMD) on 9 row-blocks of 128 rows; the first
k0 = ceil(cnt0/1024) blocks hold label-0 rows, the rest label-1 rows. The
9*128*8 = 9216 row slots cover the 8192 real rows; padded slots are duplicate
rows with weight 0.
"""

import math
import os
from contextlib import ExitStack

# concourse imports jax; make sure a cpu-pinned env doesn't hide the
# neuron devices if jax hasn't been imported yet.
if os.environ.get("JAX_PLATFORMS") == "cpu":
    del os.environ["JAX_PLATFORMS"]

import numpy as np
import ml_dtypes

import concourse.tile as tile
from concourse import bacc, mybir
from concourse.bass_utils import run_bass_kernel_spmd

N = 8192
D = 128
NCORES = 8
RB = 9                     # row-blocks per core
PBLK = 128                 # rows per block
SLOTS = RB * PBLK          # row slots per core
TEMPERATURE = 0.1
BASE_TEMPERATURE = 0.07
PS_W = 2048                # psum tile width (4 banks), f32
MM_W = 512                 # max matmul free dim (one psum bank of f32)
FTB_W = 2048               # ftb column-tile width


def _cuts(c0, c1, step, extra=(), lead=None):
    """Split [c0,c1) at multiples of `step` relative to c0 (or after an
    optional smaller `lead` first piece), plus extra absolute cut points."""
    cuts = {c0, c1}
    a = c0 + lead if lead else c0
    cuts |= set(range(a, c1, step))
    cuts |= {c for c in extra if c0 < c < c1}
    cuts = sorted(cuts)
    return [(x, y - x) for x, y in zip(cuts, cuts[1:])]


def _build(cnt0):
    """Emit the SPMD program for one core, parametrized by the label split."""
    cnt1 = N - cnt0
    k0 = math.ceil(cnt0 / 1024)
    f32 = mybir.dt.float32
    bf16 = mybir.dt.bfloat16
    Exp = mybir.ActivationFunctionType.Exp
    Ln = mybir.ActivationFunctionType.Ln
    X = mybir.AxisListType.X
    ADD = mybir.AluOpType.add

    posmax = max(cnt0, cnt1, 1)

    nc = bacc.Bacc("TRN2", target_bir_lowering=False, debug=False,
                   enable_asserts=False, num_devices=NCORES)

    ftb_d = nc.dram_tensor("ftb", [D, N], bf16, kind="ExternalInput").ap()
    ftr_d = nc.dram_tensor("ftr", [D, SLOTS], bf16, kind="ExternalInput").ap()
    w_d = nc.dram_tensor("w", [PBLK, RB], f32, kind="ExternalInput").ap()
    out_d = nc.dram_tensor("out", [PBLK, RB], f32, kind="ExternalOutput").ap()

    with tile.TileContext(nc) as tc, ExitStack() as ctx:
        io = ctx.enter_context(tc.tile_pool(name="io", bufs=1))
        ftbp = ctx.enter_context(tc.tile_pool(name="ftbp", bufs=N // FTB_W))
        psp = ctx.enter_context(tc.tile_pool(name="psp", bufs=2, space="PSUM"))
        scr = ctx.enter_context(tc.tile_pool(name="scr", bufs=2))
        rowp = ctx.enter_context(tc.tile_pool(name="rowp", bufs=2))

        # --- inputs resident in SBUF ---
        # ftb segments anchored at rb0's first-needed range (its negative
        # range), with a small 512-wide lead segment so the first matmul can
        # start as soon as ~160KB have landed. Loads spread over 3 queues.
        rb0neg = (cnt0, N) if k0 > 0 else (0, cnt0)
        rb0pos = (0, cnt0) if k0 > 0 else (cnt0, N)
        segs = []
        for (a, b), lead in ((rb0neg, 512), (rb0pos, None)):
            if b > a:
                segs.extend(_cuts(a, b, FTB_W, lead=lead if b - a > 512 else None))

        # first the lead segment + rb0's lhsT slice, then everything else
        ftr0 = io.tile([D, PBLK], bf16)
        nc.sync.dma_start(out=ftr0[:], in_=ftr_d[:, 0:PBLK])
        engines = [nc.gpsimd, nc.sync, nc.scalar]
        ftbt = []
        for j, (g0, w) in enumerate(segs):
            fb = ftbp.tile([D, w], bf16, name=f"ftb{j}", tag=f"ftb{j}")
            engines[j % len(engines)].dma_start(out=fb[:], in_=ftb_d[:, g0:g0 + w])
            ftbt.append((g0, w, fb))
        seg_bounds = [g0 for g0, _, _ in ftbt]
        ftr1 = io.tile([D, SLOTS - PBLK], bf16)
        nc.gpsimd.dma_start(out=ftr1[:], in_=ftr_d[:, PBLK:])
        wt = io.tile([PBLK, RB], f32)
        nc.sync.dma_start(out=wt[:], in_=w_d[:])
        b10 = io.tile([PBLK, 1], f32)
        nc.vector.memset(b10[:], -10.0)
        b10p = io.tile([PBLK, 1], f32)
        nc.vector.memset(b10p[:], 10.0)
        cout = io.tile([PBLK, RB], f32)
        nc.vector.memset(cout[:], 0.0)

        def mm_range(lhsT, c0, c1, act_fn, lead=None):
            """Matmul columns [c0,c1) into psum tiles; per psum tile call
            act_fn(ps, width, tile_index)."""
            for ti, (t0, tw) in enumerate(_cuts(c0, c1, PS_W, lead=lead)):
                ps = psp.tile([PBLK, PS_W], f32, name="ps", tag="ps")
                for (p0, pw) in _cuts(t0, t0 + tw, MM_W, extra=seg_bounds):
                    g0, gw, fb = next(
                        s for s in ftbt if s[0] <= p0 and p0 + pw <= s[0] + s[1])
                    nc.tensor.matmul(
                        ps[:, p0 - t0:p0 - t0 + pw],
                        lhsT,
                        fb[:, p0 - g0:p0 - g0 + pw],
                        start=True, stop=True)
                act_fn(ps, tw, ti)

        for rb in range(RB):
            is0 = rb < k0
            pos = (0, cnt0) if is0 else (cnt0, N)
            neg = (cnt0, N) if is0 else (0, cnt0)
            lhsT = ftr0[:] if rb == 0 else ftr1[:, (rb - 1) * PBLK:rb * PBLK]

            negw = neg[1] - neg[0]
            posw = pos[1] - pos[0]
            lead = 512 if rb == 0 and negw > 512 else None
            nnegt = max(1, len(_cuts(neg[0], neg[1], PS_W, lead=lead)) if negw else 1)

            # --- negative columns: exp(10r-10), row-accumulate -> NL ---
            at = rowp.tile([PBLK, nnegt], f32, name="at", tag="at")

            def neg_act(ps, tw, ti, at=at):
                ts = scr.tile([PBLK, PS_W], f32, name="ts", tag="ts")
                nc.scalar.activation(ts[:, :tw], ps[:, :tw], Exp,
                                     bias=b10[:], scale=10.0,
                                     accum_out=at[:, ti:ti + 1])

            if negw:
                mm_range(lhsT, neg[0], neg[1], neg_act, lead=lead)

            nl = rowp.tile([PBLK, 1], f32, name="nl", tag="nl")
            if negw:
                nc.vector.tensor_reduce(nl[:], at[:], X, ADD)
            else:
                nc.vector.memset(nl[:], 0.0)
            l1p = rowp.tile([PBLK, 1], f32, name="l1p", tag="l1p")
            nc.scalar.activation(l1p[:], nl[:], Ln, bias=1.0, scale=1.0)

            # --- positive columns: e = exp(-l) = exp(-10r + 10); NL enters the
            # ln pass as a per-row scale, so this pass has no NL dependency ---
            tp = scr.tile([PBLK, posmax], f32, name="tp", tag="tp")

            def pos_act(ps, tw, ti, tp=tp):
                off = ti * PS_W
                nc.scalar.activation(tp[:, off:off + tw], ps[:, :tw], Exp,
                                     bias=b10p[:], scale=-10.0)

            if not posw:
                continue  # degenerate single-label case; cout stays zero
            mm_range(lhsT, pos[0], pos[1], pos_act)

            # --- sum_pos [l - ln(exp l + NL)] = -sum_pos ln(1 + NL*e) ---
            tz = scr.tile([PBLK, posmax], f32, name="tz", tag="tz")
            sz = rowp.tile([PBLK, 1], f32, name="sz", tag="sz")
            nc.scalar.activation(tz[:, :posw], tp[:, :posw], Ln,
                                 bias=1.0, scale=nl[:], accum_out=sz[:])

            # --- row combine: S = log1p(NL) - sz; out = S*w (w negated) ---
            s2 = rowp.tile([PBLK, 1], f32, name="s2", tag="s2")
            nc.vector.tensor_sub(s2[:], sz[:], l1p[:])
            nc.vector.tensor_mul(cout[:, rb:rb + 1], s2[:], wt[:, rb:rb + 1])

        nc.sync.dma_start(out=out_d[:], in_=cout[:])

    _pin_act_table(nc)
    nc.compile()
    return nc


def _pin_act_table(nc, set_name="natural_log_exp_and_others"):
    """The default table chooser alternates exp-only/ln-only sets, paying a
    ~1.3us ACT_TABLE_LOAD on every Exp<->Ln transition. All our activation
    funcs live in one named set; offer the pass only that set (with a
    coverage check so a future func change falls back safely)."""
    from concourse.hw_specs import get_activation_tables
    from concourse import _compat  # noqa: F401
    import concourse.bacc as bacc_mod

    used = {
        inst.func
        for b in nc.main_func.blocks
        for inst in b.instructions
        if isinstance(inst, mybir.InstActivation)
    }
    tables = get_activation_tables(nc.m.arch)
    pinned = tables.get(set_name)
    if pinned is None or not used.issubset(pinned):
        return  # fall back to default behavior

    # act_func_set_id is the INDEX into act_info.json's act_func_sets, so the
    # list must keep its length/order; empty the other sets so the chooser
    # can only pick ours.
    masked = [(n, f if n == set_name else set()) for n, f in tables.items()]

    def patched():
        bacc_mod._bass_rust.insert_act_table_loads(nc, masked)

    nc.insert_act_table_loads = patched


def _prepare(feats, labels):
    """Host-side sharding prep: sort by label, build per-core inputs."""
    F = np.ascontiguousarray(np.asarray(feats, dtype=np.float32).reshape(N, D))
    lab = np.asarray(labels).reshape(N)
    perm = np.argsort(lab, kind="stable")
    Fs = F[perm]
    ys = lab[perm]
    cnt0 = int(np.searchsorted(ys, 1))
    cnt1 = N - cnt0
    k0 = math.ceil(cnt0 / 1024)

    ftb = np.ascontiguousarray(Fs.T.astype(ml_dtypes.bfloat16))  # [D, N]

    # global slot lists (indices into sorted order) + weights
    n0slots = NCORES * k0 * PBLK
    n1slots = NCORES * (RB - k0) * PBLK
    idx0 = np.full(n0slots, max(cnt0 - 1, 0), dtype=np.int64)
    idx0[:cnt0] = np.arange(cnt0)
    w0 = np.zeros(n0slots, dtype=np.float32)
    if cnt0 > 1:
        w0[:cnt0] = -1.0 / (cnt0 - 1)  # negated: device emits -S per row
    idx1 = np.full(n1slots, N - 1, dtype=np.int64)
    idx1[:cnt1] = cnt0 + np.arange(cnt1)
    w1 = np.zeros(n1slots, dtype=np.float32)
    if cnt1 > 1:
        w1[:cnt1] = -1.0 / (cnt1 - 1)

    idx0 = idx0.reshape(NCORES, k0 * PBLK)
    w0 = w0.reshape(NCORES, k0 * PBLK)
    idx1 = idx1.reshape(NCORES, (RB - k0) * PBLK)
    w1 = w1.reshape(NCORES, (RB - k0) * PBLK)

    in_maps = []
    for c in range(NCORES):
        rows = np.concatenate([idx0[c], idx1[c]])
        ftr = np.ascontiguousarray(Fs[rows].T.astype(ml_dtypes.bfloat16))
        wc = np.concatenate([w0[c], w1[c]]).reshape(RB, PBLK).T.copy()
        in_maps.append({"ftb": ftb, "ftr": ftr, "w": wc})
    return cnt0, in_maps


def _assemble(results):
    total = 0.0
    for r in results:
        total += float(r["out"].astype(np.float64).sum())
    return np.float32(-(TEMPERATURE / BASE_TEMPERATURE) * total / N)


_CACHE = {}


def kernel(feats, labels):
    cnt0, in_maps = _prepare(feats, labels)
    nc = _CACHE.get(cnt0)
    if nc is None:
        nc = _CACHE[cnt0] = _build(cnt0)
    res = run_bass_kernel_spmd(nc, in_maps, list(range(NCORES))).results
    return _assemble(res)


# revision 26
# speedup vs baseline: 1.1723x; 1.0018x over previous
"""PixelContrastLoss forward on 8 Trainium2 cores.

Contract: kernel(feats [8192,1,128] f32, labels [8192] i32) -> scalar f32 loss,
matching:
    logits = (F @ F.T) / T;  l = logits - rowmax
    NL_i = sum_{j: label_j != label_i} exp(l_ij)
    loss = -(T/BT) * mean_i [ sum_{j in pos, j!=i} (l_ij - log(exp(l_ij)+NL_i))
                              / (cnt_i - 1) ]

Math used by the kernel:
  * l - log(exp l + NL) is invariant to the rowmax shift, so we shift by the
    constant 10 (= diag logit of L2-normalized rows / T) instead of computing
    the row max.
  * Rows/columns are sorted by label on the host (loss is invariant under
    permutation), so each row's positive/negative column sets are the two
    compile-time ranges [0,cnt0) and [cnt0,N).
  * sum_pos l = 10*sum_pos r - 10*cntA comes from plain row-sums of the raw
    logits r (DVE reduce of PSUM), no elementwise pass.
  * diag correction: l_ii ~= 0, so subtracting the self-pair contributes
    +log1p(NL_i) to the row sum.

Each core runs the same program (SPMD) on 9 row-blocks of 128 rows; the first
k0 = ceil(cnt0/1024) blocks hold label-0 rows, the rest label-1 rows. The
9*128*8 = 9216 row slots cover the 8192 real rows; padded slots are duplicate
rows with weight 0.
"""

import math
import os
from contextlib import ExitStack

# concourse imports jax; make sure a cpu-pinned env doesn't hide the
# neuron devices if jax hasn't been imported yet.
if os.environ.get("JAX_PLATFORMS") == "cpu":
    del os.environ["JAX_PLATFORMS"]

import numpy as np
import ml_dtypes

import concourse.tile as tile
from concourse import bacc, mybir
from concourse.bass_utils import run_bass_kernel_spmd

N = 8192
D = 128
NCORES = 8
RB = 9                     # row-blocks per core
PBLK = 128                 # rows per block
SLOTS = RB * PBLK          # row slots per core
TEMPERATURE = 0.1
BASE_TEMPERATURE = 0.07
PS_W = 2048                # psum tile width (4 banks), f32
MM_W = 512                 # max matmul free dim (one psum bank of f32)
FTB_W = 2048               # ftb column-tile width


def _cuts(c0, c1, step, extra=(), lead=None):
    """Split [c0,c1) at multiples of `step` relative to c0 (or after an
    optional smaller `lead` first piece), plus extra absolute cut points."""
    cuts = {c0, c1}
    a = c0 + lead if lead else c0
    cuts |= set(range(a, c1, step))
    cuts |= {c for c in extra if c0 < c < c1}
    cuts = sorted(cuts)
    return [(x, y - x) for x, y in zip(cuts, cuts[1:])]


def _build(cnt0):
    """Emit the SPMD program for one core, parametrized by the label split."""
    cnt1 = N - cnt0
    k0 = math.ceil(cnt0 / 1024)
    f32 = mybir.dt.float32
    bf16 = mybir.dt.bfloat16
    Exp = mybir.ActivationFunctionType.Exp
    Ln = mybir.ActivationFunctionType.Ln
    X = mybir.AxisListType.X
    ADD = mybir.AluOpType.add

    posmax = max(cnt0, cnt1, 1)

    nc = bacc.Bacc("TRN2", target_bir_lowering=False, debug=False,
                   enable_asserts=False, num_devices=NCORES)

    ftb_d = nc.dram_tensor("ftb", [D, N], bf16, kind="ExternalInput").ap()
    ftr_d = nc.dram_tensor("ftr", [D, SLOTS], bf16, kind="ExternalInput").ap()
    w_d = nc.dram_tensor("w", [PBLK, RB], f32, kind="ExternalInput").ap()
    out_d = nc.dram_tensor("out", [PBLK, RB], f32, kind="ExternalOutput").ap()

    with tile.TileContext(nc) as tc, ExitStack() as ctx:
        io = ctx.enter_context(tc.tile_pool(name="io", bufs=1))
        ftbp = ctx.enter_context(tc.tile_pool(name="ftbp", bufs=N // FTB_W))
        psp = ctx.enter_context(tc.tile_pool(name="psp", bufs=2, space="PSUM"))
        scr = ctx.enter_context(tc.tile_pool(name="scr", bufs=2))
        rowp = ctx.enter_context(tc.tile_pool(name="rowp", bufs=2))

        # --- inputs resident in SBUF ---
        # ftb segments anchored at rb0's first-needed range (its negative
        # range), with a small 512-wide lead segment so the first matmul can
        # start as soon as ~160KB have landed. Loads spread over 3 queues.
        rb0neg = (cnt0, N) if k0 > 0 else (0, cnt0)
        rb0pos = (0, cnt0) if k0 > 0 else (cnt0, N)
        segs = []
        for (a, b), lead in ((rb0neg, 512), (rb0pos, None)):
            if b > a:
                segs.extend(_cuts(a, b, FTB_W, lead=lead if b - a > 512 else None))

        # first the lead segment + rb0's lhsT slice, then everything else
        ftr0 = io.tile([D, PBLK], bf16)
        nc.sync.dma_start(out=ftr0[:], in_=ftr_d[:, 0:PBLK])
        engines = [nc.gpsimd, nc.sync, nc.scalar]
        ftbt = []
        for j, (g0, w) in enumerate(segs):
            fb = ftbp.tile([D, w], bf16, name=f"ftb{j}", tag=f"ftb{j}")
            engines[j % len(engines)].dma_start(out=fb[:], in_=ftb_d[:, g0:g0 + w])
            ftbt.append((g0, w, fb))
        seg_bounds = [g0 for g0, _, _ in ftbt]
        ftr1 = io.tile([D, SLOTS - PBLK], bf16)
        nc.gpsimd.dma_start(out=ftr1[:], in_=ftr_d[:, PBLK:])
        wt = io.tile([PBLK, RB], f32)
        nc.sync.dma_start(out=wt[:], in_=w_d[:])
        b10 = io.tile([PBLK, 1], f32)
        nc.vector.memset(b10[:], -10.0)
        b10p = io.tile([PBLK, 1], f32)
        nc.vector.memset(b10p[:], 10.0)
        cout = io.tile([PBLK, RB], f32)
        nc.vector.memset(cout[:], 0.0)

        def mm_range(lhsT, c0, c1, act_fn, lead=None):
            """Matmul columns [c0,c1) into psum tiles; per psum tile call
            act_fn(ps, width, tile_index)."""
            for ti, (t0, tw) in enumerate(_cuts(c0, c1, PS_W, lead=lead)):
                ps = psp.tile([PBLK, PS_W], f32, name="ps", tag="ps")
                for (p0, pw) in _cuts(t0, t0 + tw, MM_W, extra=seg_bounds):
                    g0, gw, fb = next(
                        s for s in ftbt if s[0] <= p0 and p0 + pw <= s[0] + s[1])
                    nc.tensor.matmul(
                        ps[:, p0 - t0:p0 - t0 + pw],
                        lhsT,
                        fb[:, p0 - g0:p0 - g0 + pw],
                        start=True, stop=True)
                act_fn(ps, tw, ti)

        for rb in range(RB):
            is0 = rb < k0
            pos = (0, cnt0) if is0 else (cnt0, N)
            neg = (cnt0, N) if is0 else (0, cnt0)
            lhsT = ftr0[:] if rb == 0 else ftr1[:, (rb - 1) * PBLK:rb * PBLK]

            negw = neg[1] - neg[0]
            posw = pos[1] - pos[0]
            lead = 512 if rb == 0 and negw > 512 else None
            nnegt = max(1, len(_cuts(neg[0], neg[1], PS_W, lead=lead)) if negw else 1)

            # --- negative columns: exp(10r-10), row-accumulate -> NL ---
            at = rowp.tile([PBLK, nnegt], f32, name="at", tag="at")

            def neg_act(ps, tw, ti, at=at):
                ts = scr.tile([PBLK, PS_W], f32, name="ts", tag="ts")
                nc.scalar.activation(ts[:, :tw], ps[:, :tw], Exp,
                                     bias=b10[:], scale=10.0,
                                     accum_out=at[:, ti:ti + 1])

            if negw:
                mm_range(lhsT, neg[0], neg[1], neg_act, lead=lead)

            nl = rowp.tile([PBLK, 1], f32, name="nl", tag="nl")
            if negw:
                nc.vector.tensor_reduce(nl[:], at[:], X, ADD)
            else:
                nc.vector.memset(nl[:], 0.0)

            # --- positive columns: e = exp(-l) = exp(-10r + 10); NL enters the
            # ln pass as a per-row scale, so this pass has no NL dependency ---
            tp = scr.tile([PBLK, posmax], f32, name="tp", tag="tp")

            def pos_act(ps, tw, ti, tp=tp):
                off = ti * PS_W
                nc.scalar.activation(tp[:, off:off + tw], ps[:, :tw], Exp,
                                     bias=b10p[:], scale=-10.0)

            if not posw:
                continue  # degenerate single-label case; cout stays zero
            mm_range(lhsT, pos[0], pos[1], pos_act)

            # emitted after the pos exps so its NL dependency never bubbles ACT
            l1p = rowp.tile([PBLK, 1], f32, name="l1p", tag="l1p")
            nc.scalar.activation(l1p[:], nl[:], Ln, bias=1.0, scale=1.0)

            # --- sum_pos [l - ln(exp l + NL)] = -sum_pos ln(1 + NL*e) ---
            tz = scr.tile([PBLK, posmax], f32, name="tz", tag="tz")
            sz = rowp.tile([PBLK, 1], f32, name="sz", tag="sz")
            nc.scalar.activation(tz[:, :posw], tp[:, :posw], Ln,
                                 bias=1.0, scale=nl[:], accum_out=sz[:])

            # --- row combine: S = log1p(NL) - sz; out = S*w (w negated) ---
            s2 = rowp.tile([PBLK, 1], f32, name="s2", tag="s2")
            nc.vector.tensor_sub(s2[:], sz[:], l1p[:])
            nc.vector.tensor_mul(cout[:, rb:rb + 1], s2[:], wt[:, rb:rb + 1])
            if rb == RB - 2:
                nc.sync.dma_start(out=out_d[:, :RB - 1], in_=cout[:, :RB - 1])

        nc.sync.dma_start(out=out_d[:, RB - 1:], in_=cout[:, RB - 1:])

    _pin_act_table(nc)
    nc.compile()
    return nc


def _pin_act_table(nc, set_name="natural_log_exp_and_others"):
    """The default table chooser alternates exp-only/ln-only sets, paying a
    ~1.3us ACT_TABLE_LOAD on every Exp<->Ln transition. All our activation
    funcs live in one named set; offer the pass only that set (with a
    coverage check so a future func change falls back safely)."""
    from concourse.hw_specs import get_activation_tables
    from concourse import _compat  # noqa: F401
    import concourse.bacc as bacc_mod

    used = {
        inst.func
        for b in nc.main_func.blocks
        for inst in b.instructions
        if isinstance(inst, mybir.InstActivation)
    }
    tables = get_activation_tables(nc.m.arch)
    pinned = tables.get(set_name)
    if pinned is None or not used.issubset(pinned):
        return  # fall back to default behavior

    # act_func_set_id is the INDEX into act_info.json's act_func_sets, so the
    # list must keep its length/order; empty the other sets so the chooser
    # can only pick ours.
    masked = [(n, f if n == set_name else set()) for n, f in tables.items()]

    def patched():
        bacc_mod._bass_rust.insert_act_table_loads(nc, masked)

    nc.insert_act_table_loads = patched


def _prepare(feats, labels):
    """Host-side sharding prep: sort by label, build per-core inputs."""
    F = np.ascontiguousarray(np.asarray(feats, dtype=np.float32).reshape(N, D))
    lab = np.asarray(labels).reshape(N)
    perm = np.argsort(lab, kind="stable")
    Fs = F[perm]
    ys = lab[perm]
    cnt0 = int(np.searchsorted(ys, 1))
    cnt1 = N - cnt0
    k0 = math.ceil(cnt0 / 1024)

    ftb = np.ascontiguousarray(Fs.T.astype(ml_dtypes.bfloat16))  # [D, N]

    # global slot lists (indices into sorted order) + weights
    n0slots = NCORES * k0 * PBLK
    n1slots = NCORES * (RB - k0) * PBLK
    idx0 = np.full(n0slots, max(cnt0 - 1, 0), dtype=np.int64)
    idx0[:cnt0] = np.arange(cnt0)
    w0 = np.zeros(n0slots, dtype=np.float32)
    if cnt0 > 1:
        w0[:cnt0] = -1.0 / (cnt0 - 1)  # negated: device emits -S per row
    idx1 = np.full(n1slots, N - 1, dtype=np.int64)
    idx1[:cnt1] = cnt0 + np.arange(cnt1)
    w1 = np.zeros(n1slots, dtype=np.float32)
    if cnt1 > 1:
        w1[:cnt1] = -1.0 / (cnt1 - 1)

    idx0 = idx0.reshape(NCORES, k0 * PBLK)
    w0 = w0.reshape(NCORES, k0 * PBLK)
    idx1 = idx1.reshape(NCORES, (RB - k0) * PBLK)
    w1 = w1.reshape(NCORES, (RB - k0) * PBLK)

    in_maps = []
    for c in range(NCORES):
        rows = np.concatenate([idx0[c], idx1[c]])
        ftr = np.ascontiguousarray(Fs[rows].T.astype(ml_dtypes.bfloat16))
        wc = np.concatenate([w0[c], w1[c]]).reshape(RB, PBLK).T.copy()
        in_maps.append({"ftb": ftb, "ftr": ftr, "w": wc})
    return cnt0, in_maps


def _assemble(results):
    total = 0.0
    for r in results:
        total += float(r["out"].astype(np.float64).sum())
    return np.float32(-(TEMPERATURE / BASE_TEMPERATURE) * total / N)


_CACHE = {}


def kernel(feats, labels):
    cnt0, in_maps = _prepare(feats, labels)
    nc = _CACHE.get(cnt0)
    if nc is None:
        nc = _CACHE[cnt0] = _build(cnt0)
    res = run_bass_kernel_spmd(nc, in_maps, list(range(NCORES))).results
    return _assemble(res)


# revision 28
# speedup vs baseline: 1.1802x; 1.0067x over previous
"""PixelContrastLoss forward on 8 Trainium2 cores.

Contract: kernel(feats [8192,1,128] f32, labels [8192] i32) -> scalar f32 loss,
matching:
    logits = (F @ F.T) / T;  l = logits - rowmax
    NL_i = sum_{j: label_j != label_i} exp(l_ij)
    loss = -(T/BT) * mean_i [ sum_{j in pos, j!=i} (l_ij - log(exp(l_ij)+NL_i))
                              / (cnt_i - 1) ]

Math used by the kernel:
  * l - log(exp l + NL) is invariant to the rowmax shift, so we shift by the
    constant 10 (= diag logit of L2-normalized rows / T) instead of computing
    the row max.
  * Rows/columns are sorted by label on the host (loss is invariant under
    permutation), so each row's positive/negative column sets are the two
    compile-time ranges [0,cnt0) and [cnt0,N).
  * sum_pos l = 10*sum_pos r - 10*cntA comes from plain row-sums of the raw
    logits r (DVE reduce of PSUM), no elementwise pass.
  * diag correction: l_ii ~= 0, so subtracting the self-pair contributes
    +log1p(NL_i) to the row sum.

Each core runs the same program (SPMD) on 9 row-blocks of 128 rows; the first
k0 = ceil(cnt0/1024) blocks hold label-0 rows, the rest label-1 rows. The
9*128*8 = 9216 row slots cover the 8192 real rows; padded slots are duplicate
rows with weight 0.
"""

import math
import os
from contextlib import ExitStack

# concourse imports jax; make sure a cpu-pinned env doesn't hide the
# neuron devices if jax hasn't been imported yet.
if os.environ.get("JAX_PLATFORMS") == "cpu":
    del os.environ["JAX_PLATFORMS"]

import numpy as np
import ml_dtypes

import concourse.tile as tile
from concourse import bacc, mybir
from concourse.bass_utils import run_bass_kernel_spmd

N = 8192
D = 128
NCORES = 8
RB = 9                     # row-blocks per core
PBLK = 128                 # rows per block
SLOTS = RB * PBLK          # row slots per core
TEMPERATURE = 0.1
BASE_TEMPERATURE = 0.07
PS_W = 2048                # psum tile width (4 banks), f32
MM_W = 512                 # max matmul free dim (one psum bank of f32)
FTB_W = 2048               # ftb column-tile width


def _cuts(c0, c1, step, extra=(), lead=None):
    """Split [c0,c1) at multiples of `step` relative to c0 (or after an
    optional smaller `lead` first piece), plus extra absolute cut points."""
    cuts = {c0, c1}
    a = c0 + lead if lead else c0
    cuts |= set(range(a, c1, step))
    cuts |= {c for c in extra if c0 < c < c1}
    cuts = sorted(cuts)
    return [(x, y - x) for x, y in zip(cuts, cuts[1:])]


def _build(cnt0):
    """Emit the SPMD program for one core, parametrized by the label split."""
    cnt1 = N - cnt0
    k0 = math.ceil(cnt0 / 1024)
    f32 = mybir.dt.float32
    bf16 = mybir.dt.bfloat16
    Exp = mybir.ActivationFunctionType.Exp
    Ln = mybir.ActivationFunctionType.Ln
    X = mybir.AxisListType.X
    ADD = mybir.AluOpType.add

    posmax = max(cnt0, cnt1, 1)

    nc = bacc.Bacc("TRN2", target_bir_lowering=False, debug=False,
                   enable_asserts=False, num_devices=NCORES)

    ftb_d = nc.dram_tensor("ftb", [D, N], bf16, kind="ExternalInput").ap()
    ftr_d = nc.dram_tensor("ftr", [D, SLOTS], bf16, kind="ExternalInput").ap()
    w_d = nc.dram_tensor("w", [PBLK, RB], f32, kind="ExternalInput").ap()
    out_d = nc.dram_tensor("out", [PBLK, RB], f32, kind="ExternalOutput").ap()

    with tile.TileContext(nc) as tc, ExitStack() as ctx:
        io = ctx.enter_context(tc.tile_pool(name="io", bufs=1))
        ftbp = ctx.enter_context(tc.tile_pool(name="ftbp", bufs=N // FTB_W))
        psp = ctx.enter_context(tc.tile_pool(name="psp", bufs=2, space="PSUM"))
        scr = ctx.enter_context(tc.tile_pool(name="scr", bufs=2))
        rowp = ctx.enter_context(tc.tile_pool(name="rowp", bufs=2))

        # --- inputs resident in SBUF ---
        # ftb segments anchored at rb0's first-needed range (its negative
        # range), with a small 512-wide lead segment so the first matmul can
        # start as soon as ~160KB have landed. Loads spread over 3 queues.
        rb0neg = (cnt0, N) if k0 > 0 else (0, cnt0)
        rb0pos = (0, cnt0) if k0 > 0 else (cnt0, N)
        segs = []
        for (a, b), lead in ((rb0neg, 512), (rb0pos, None)):
            if b > a:
                segs.extend(_cuts(a, b, FTB_W, lead=lead if b - a > 512 else None))

        # first the lead segment + rb0's lhsT slice, then everything else
        ftr0 = io.tile([D, PBLK], bf16)
        nc.sync.dma_start(out=ftr0[:], in_=ftr_d[:, 0:PBLK])
        engines = [nc.gpsimd, nc.sync, nc.scalar]
        ftbt = []
        for j, (g0, w) in enumerate(segs):
            fb = ftbp.tile([D, w], bf16, name=f"ftb{j}", tag=f"ftb{j}")
            engines[j % len(engines)].dma_start(out=fb[:], in_=ftb_d[:, g0:g0 + w])
            ftbt.append((g0, w, fb))
        seg_bounds = [g0 for g0, _, _ in ftbt]
        ftr1 = io.tile([D, SLOTS - PBLK], bf16)
        nc.gpsimd.dma_start(out=ftr1[:], in_=ftr_d[:, PBLK:])
        wt = io.tile([PBLK, RB], f32)
        nc.sync.dma_start(out=wt[:], in_=w_d[:])
        b10 = io.tile([PBLK, 1], f32)
        nc.vector.memset(b10[:], -10.0)
        b10p = io.tile([PBLK, 1], f32)
        nc.vector.memset(b10p[:], 10.0)
        cout = io.tile([PBLK, RB], f32)
        nc.vector.memset(cout[:], 0.0)

        # PE warm-up during the input-DMA lead-in: dummy matmuls on
        # uninitialized SBUF (result never read) so the HAM clock-gate opens
        # before the real matmuls start.
        wl = io.tile([D, PBLK], bf16)
        nc.vector.memset(wl[:], 0.0)
        wr = io.tile([D, MM_W], bf16)
        nc.vector.memset(wr[:], 0.0)
        wps = psp.tile([PBLK, PS_W], f32, name="wps", tag="ps")
        for _ in range(10):
            nc.tensor.matmul(wps[:, :MM_W], wl[:], wr[:],
                             start=True, stop=True, skip_group_check=True)

        def mm_range(lhsT, c0, c1, act_fn, lead=None):
            """Matmul columns [c0,c1) into psum tiles; per psum tile call
            act_fn(ps, width, tile_index)."""
            for ti, (t0, tw) in enumerate(_cuts(c0, c1, PS_W, lead=lead)):
                ps = psp.tile([PBLK, PS_W], f32, name="ps", tag="ps")
                for (p0, pw) in _cuts(t0, t0 + tw, MM_W, extra=seg_bounds):
                    g0, gw, fb = next(
                        s for s in ftbt if s[0] <= p0 and p0 + pw <= s[0] + s[1])
                    nc.tensor.matmul(
                        ps[:, p0 - t0:p0 - t0 + pw],
                        lhsT,
                        fb[:, p0 - g0:p0 - g0 + pw],
                        start=True, stop=True)
                act_fn(ps, tw, ti)

        for rb in range(RB):
            is0 = rb < k0
            pos = (0, cnt0) if is0 else (cnt0, N)
            neg = (cnt0, N) if is0 else (0, cnt0)
            lhsT = ftr0[:] if rb == 0 else ftr1[:, (rb - 1) * PBLK:rb * PBLK]

            negw = neg[1] - neg[0]
            posw = pos[1] - pos[0]
            lead = 512 if rb == 0 and negw > 512 else None
            nnegt = max(1, len(_cuts(neg[0], neg[1], PS_W, lead=lead)) if negw else 1)

            # --- negative columns: exp(10r-10), row-accumulate -> NL ---
            at = rowp.tile([PBLK, nnegt], f32, name="at", tag="at")

            def neg_act(ps, tw, ti, at=at):
                ts = scr.tile([PBLK, PS_W], f32, name="ts", tag="ts")
                nc.scalar.activation(ts[:, :tw], ps[:, :tw], Exp,
                                     bias=b10[:], scale=10.0,
                                     accum_out=at[:, ti:ti + 1])

            if negw:
                mm_range(lhsT, neg[0], neg[1], neg_act, lead=lead)

            nl = rowp.tile([PBLK, 1], f32, name="nl", tag="nl")
            if negw:
                nc.vector.tensor_reduce(nl[:], at[:], X, ADD)
            else:
                nc.vector.memset(nl[:], 0.0)

            # --- positive columns: e = exp(-l) = exp(-10r + 10); NL enters the
            # ln pass as a per-row scale, so this pass has no NL dependency ---
            tp = scr.tile([PBLK, posmax], f32, name="tp", tag="tp")

            def pos_act(ps, tw, ti, tp=tp):
                off = ti * PS_W
                nc.scalar.activation(tp[:, off:off + tw], ps[:, :tw], Exp,
                                     bias=b10p[:], scale=-10.0)

            if not posw:
                continue  # degenerate single-label case; cout stays zero
            mm_range(lhsT, pos[0], pos[1], pos_act)

            # emitted after the pos exps so its NL dependency never bubbles ACT
            l1p = rowp.tile([PBLK, 1], f32, name="l1p", tag="l1p")
            nc.scalar.activation(l1p[:], nl[:], Ln, bias=1.0, scale=1.0)

            # --- sum_pos [l - ln(exp l + NL)] = -sum_pos ln(1 + NL*e) ---
            tz = scr.tile([PBLK, posmax], f32, name="tz", tag="tz")
            sz = rowp.tile([PBLK, 1], f32, name="sz", tag="sz")
            nc.scalar.activation(tz[:, :posw], tp[:, :posw], Ln,
                                 bias=1.0, scale=nl[:], accum_out=sz[:])

            # --- row combine: S = log1p(NL) - sz; out = S*w (w negated) ---
            s2 = rowp.tile([PBLK, 1], f32, name="s2", tag="s2")
            nc.vector.tensor_sub(s2[:], sz[:], l1p[:])
            nc.vector.tensor_mul(cout[:, rb:rb + 1], s2[:], wt[:, rb:rb + 1])
            if rb == RB - 2:
                nc.sync.dma_start(out=out_d[:, :RB - 1], in_=cout[:, :RB - 1])

        nc.sync.dma_start(out=out_d[:, RB - 1:], in_=cout[:, RB - 1:])

    _pin_act_table(nc)
    nc.compile()
    return nc


def _pin_act_table(nc, set_name="natural_log_exp_and_others"):
    """The default table chooser alternates exp-only/ln-only sets, paying a
    ~1.3us ACT_TABLE_LOAD on every Exp<->Ln transition. All our activation
    funcs live in one named set; offer the pass only that set (with a
    coverage check so a future func change falls back safely)."""
    from concourse.hw_specs import get_activation_tables
    from concourse import _compat  # noqa: F401
    import concourse.bacc as bacc_mod

    used = {
        inst.func
        for b in nc.main_func.blocks
        for inst in b.instructions
        if isinstance(inst, mybir.InstActivation)
    }
    tables = get_activation_tables(nc.m.arch)
    pinned = tables.get(set_name)
    if pinned is None or not used.issubset(pinned):
        return  # fall back to default behavior

    # act_func_set_id is the INDEX into act_info.json's act_func_sets, so the
    # list must keep its length/order; empty the other sets so the chooser
    # can only pick ours.
    masked = [(n, f if n == set_name else set()) for n, f in tables.items()]

    def patched():
        bacc_mod._bass_rust.insert_act_table_loads(nc, masked)

    nc.insert_act_table_loads = patched


def _prepare(feats, labels):
    """Host-side sharding prep: sort by label, build per-core inputs."""
    F = np.ascontiguousarray(np.asarray(feats, dtype=np.float32).reshape(N, D))
    lab = np.asarray(labels).reshape(N)
    perm = np.argsort(lab, kind="stable")
    Fs = F[perm]
    ys = lab[perm]
    cnt0 = int(np.searchsorted(ys, 1))
    cnt1 = N - cnt0
    k0 = math.ceil(cnt0 / 1024)

    ftb = np.ascontiguousarray(Fs.T.astype(ml_dtypes.bfloat16))  # [D, N]

    # global slot lists (indices into sorted order) + weights
    n0slots = NCORES * k0 * PBLK
    n1slots = NCORES * (RB - k0) * PBLK
    idx0 = np.full(n0slots, max(cnt0 - 1, 0), dtype=np.int64)
    idx0[:cnt0] = np.arange(cnt0)
    w0 = np.zeros(n0slots, dtype=np.float32)
    if cnt0 > 1:
        w0[:cnt0] = -1.0 / (cnt0 - 1)  # negated: device emits -S per row
    idx1 = np.full(n1slots, N - 1, dtype=np.int64)
    idx1[:cnt1] = cnt0 + np.arange(cnt1)
    w1 = np.zeros(n1slots, dtype=np.float32)
    if cnt1 > 1:
        w1[:cnt1] = -1.0 / (cnt1 - 1)

    idx0 = idx0.reshape(NCORES, k0 * PBLK)
    w0 = w0.reshape(NCORES, k0 * PBLK)
    idx1 = idx1.reshape(NCORES, (RB - k0) * PBLK)
    w1 = w1.reshape(NCORES, (RB - k0) * PBLK)

    in_maps = []
    for c in range(NCORES):
        rows = np.concatenate([idx0[c], idx1[c]])
        ftr = np.ascontiguousarray(Fs[rows].T.astype(ml_dtypes.bfloat16))
        wc = np.concatenate([w0[c], w1[c]]).reshape(RB, PBLK).T.copy()
        in_maps.append({"ftb": ftb, "ftr": ftr, "w": wc})
    return cnt0, in_maps


def _assemble(results):
    total = 0.0
    for r in results:
        total += float(r["out"].astype(np.float64).sum())
    return np.float32(-(TEMPERATURE / BASE_TEMPERATURE) * total / N)


_CACHE = {}


def kernel(feats, labels):
    cnt0, in_maps = _prepare(feats, labels)
    nc = _CACHE.get(cnt0)
    if nc is None:
        nc = _CACHE[cnt0] = _build(cnt0)
    res = run_bass_kernel_spmd(nc, in_maps, list(range(NCORES))).results
    return _assemble(res)


# revision 29
# speedup vs baseline: 1.2132x; 1.0280x over previous
"""PixelContrastLoss forward on 8 Trainium2 cores.

Contract: kernel(feats [8192,1,128] f32, labels [8192] i32) -> scalar f32 loss,
matching:
    logits = (F @ F.T) / T;  l = logits - rowmax
    NL_i = sum_{j: label_j != label_i} exp(l_ij)
    loss = -(T/BT) * mean_i [ sum_{j in pos, j!=i} (l_ij - log(exp(l_ij)+NL_i))
                              / (cnt_i - 1) ]

Math used by the kernel:
  * l - log(exp l + NL) is invariant to the rowmax shift, so we shift by the
    constant 10 (= diag logit of L2-normalized rows / T) instead of computing
    the row max.
  * Rows/columns are sorted by label on the host (loss is invariant under
    permutation), so each row's positive/negative column sets are the two
    compile-time ranges [0,cnt0) and [cnt0,N).
  * sum_pos l = 10*sum_pos r - 10*cntA comes from plain row-sums of the raw
    logits r (DVE reduce of PSUM), no elementwise pass.
  * diag correction: l_ii ~= 0, so subtracting the self-pair contributes
    +log1p(NL_i) to the row sum.

Each core runs the same program (SPMD) on 9 row-blocks of 128 rows; the first
k0 = ceil(cnt0/1024) blocks hold label-0 rows, the rest label-1 rows. The
9*128*8 = 9216 row slots cover the 8192 real rows; padded slots are duplicate
rows with weight 0.
"""

import math
import os
from contextlib import ExitStack

# concourse imports jax; make sure a cpu-pinned env doesn't hide the
# neuron devices if jax hasn't been imported yet.
if os.environ.get("JAX_PLATFORMS") == "cpu":
    del os.environ["JAX_PLATFORMS"]

import numpy as np
import ml_dtypes

import concourse.tile as tile
from concourse import bacc, mybir
from concourse.bass_utils import run_bass_kernel_spmd

N = 8192
D = 128
NCORES = 8
RB = 9                     # row-blocks per core
PBLK = 128                 # rows per block
SLOTS = RB * PBLK          # row slots per core
TEMPERATURE = 0.1
BASE_TEMPERATURE = 0.07
PS_W = 2048                # psum tile width (4 banks), f32
MM_W = 512                 # max matmul free dim (one psum bank of f32)
FTB_W = 2048               # ftb column-tile width


def _cuts(c0, c1, step, extra=(), lead=None):
    """Split [c0,c1) at multiples of `step` relative to c0 (or after an
    optional smaller `lead` first piece), plus extra absolute cut points."""
    cuts = {c0, c1}
    a = c0 + lead if lead else c0
    cuts |= set(range(a, c1, step))
    cuts |= {c for c in extra if c0 < c < c1}
    cuts = sorted(cuts)
    return [(x, y - x) for x, y in zip(cuts, cuts[1:])]


def _build(cnt0):
    """Emit the SPMD program for one core, parametrized by the label split."""
    cnt1 = N - cnt0
    k0 = math.ceil(cnt0 / 1024)
    f32 = mybir.dt.float32
    bf16 = mybir.dt.bfloat16
    Exp = mybir.ActivationFunctionType.Exp
    Ln = mybir.ActivationFunctionType.Ln
    X = mybir.AxisListType.X
    ADD = mybir.AluOpType.add

    posmax = max(cnt0, cnt1, 1)

    nc = bacc.Bacc("TRN2", target_bir_lowering=False, debug=False,
                   enable_asserts=False, num_devices=NCORES)

    ftb_d = nc.dram_tensor("ftb", [D, N], bf16, kind="ExternalInput").ap()
    ftr_d = nc.dram_tensor("ftr", [D, SLOTS], bf16, kind="ExternalInput").ap()
    w_d = nc.dram_tensor("w", [PBLK, RB], f32, kind="ExternalInput").ap()
    out_d = nc.dram_tensor("out", [PBLK, RB], f32, kind="ExternalOutput").ap()

    with tile.TileContext(nc) as tc, ExitStack() as ctx:
        io = ctx.enter_context(tc.tile_pool(name="io", bufs=1))
        ftbp = ctx.enter_context(tc.tile_pool(name="ftbp", bufs=N // FTB_W))
        psp = ctx.enter_context(tc.tile_pool(name="psp", bufs=2, space="PSUM"))
        scr = ctx.enter_context(tc.tile_pool(name="scr", bufs=2))
        rowp = ctx.enter_context(tc.tile_pool(name="rowp", bufs=2))

        # --- inputs resident in SBUF ---
        # ftb segments anchored at rb0's first-needed range (its negative
        # range), with a small 512-wide lead segment so the first matmul can
        # start as soon as ~160KB have landed. Loads spread over 3 queues.
        rb0neg = (cnt0, N) if k0 > 0 else (0, cnt0)
        rb0pos = (0, cnt0) if k0 > 0 else (cnt0, N)
        segs = []
        for (a, b), lead in ((rb0neg, 512), (rb0pos, None)):
            if b > a:
                segs.extend(_cuts(a, b, FTB_W, lead=lead if b - a > 512 else None))

        # first the lead segment + rb0's lhsT slice, then everything else
        ftr0 = io.tile([D, PBLK], bf16)
        nc.sync.dma_start(out=ftr0[:], in_=ftr_d[:, 0:PBLK])
        engines = [nc.gpsimd, nc.sync, nc.scalar]
        ftbt = []
        for j, (g0, w) in enumerate(segs):
            fb = ftbp.tile([D, w], bf16, name=f"ftb{j}", tag=f"ftb{j}")
            engines[j % len(engines)].dma_start(out=fb[:], in_=ftb_d[:, g0:g0 + w])
            ftbt.append((g0, w, fb))
        seg_bounds = [g0 for g0, _, _ in ftbt]
        ftr1 = io.tile([D, SLOTS - PBLK], bf16)
        nc.gpsimd.dma_start(out=ftr1[:], in_=ftr_d[:, PBLK:])
        wt = io.tile([PBLK, RB], f32)
        nc.sync.dma_start(out=wt[:], in_=w_d[:])
        b10 = io.tile([PBLK, 1], f32)
        nc.vector.memset(b10[:], -10.0)
        b10p = io.tile([PBLK, 1], f32)
        nc.vector.memset(b10p[:], 10.0)
        cout = io.tile([PBLK, RB], f32)
        nc.vector.memset(cout[:], 0.0)

        # PE warm-up during the input-DMA lead-in: dummy matmuls on
        # uninitialized SBUF (result never read) so the HAM clock-gate opens
        # before the real matmuls start.
        wl = io.tile([D, PBLK], bf16)
        nc.vector.memset(wl[:], 0.0)
        wr = io.tile([D, MM_W], bf16)
        nc.vector.memset(wr[:], 0.0)
        wps = psp.tile([PBLK, PS_W], f32, name="wps", tag="ps")
        for _ in range(10):
            nc.tensor.matmul(wps[:, :MM_W], wl[:], wr[:],
                             start=True, stop=True, skip_group_check=True)

        def mm_range(lhsT, c0, c1, act_fn, lead=None):
            """Matmul columns [c0,c1) into psum tiles; per psum tile call
            act_fn(ps, width, tile_index)."""
            for ti, (t0, tw) in enumerate(_cuts(c0, c1, PS_W, lead=lead)):
                ps = psp.tile([PBLK, PS_W], f32, name="ps", tag="ps")
                for (p0, pw) in _cuts(t0, t0 + tw, MM_W, extra=seg_bounds):
                    g0, gw, fb = next(
                        s for s in ftbt if s[0] <= p0 and p0 + pw <= s[0] + s[1])
                    nc.tensor.matmul(
                        ps[:, p0 - t0:p0 - t0 + pw],
                        lhsT,
                        fb[:, p0 - g0:p0 - g0 + pw],
                        start=True, stop=True)
                act_fn(ps, tw, ti)

        for rb in range(RB):
            is0 = rb < k0
            pos = (0, cnt0) if is0 else (cnt0, N)
            neg = (cnt0, N) if is0 else (0, cnt0)
            lhsT = ftr0[:] if rb == 0 else ftr1[:, (rb - 1) * PBLK:rb * PBLK]

            negw = neg[1] - neg[0]
            posw = pos[1] - pos[0]
            lead = 512 if rb == 0 and negw > 512 else None
            nnegt = max(1, len(_cuts(neg[0], neg[1], PS_W, lead=lead)) if negw else 1)

            # --- negative columns: exp(10r-10), row-accumulate -> NL ---
            at = rowp.tile([PBLK, nnegt], f32, name="at", tag="at")

            def neg_act(ps, tw, ti, at=at):
                ts = scr.tile([PBLK, PS_W], f32, name="ts", tag="ts")
                nc.scalar.activation(ts[:, :tw], ps[:, :tw], Exp,
                                     bias=b10[:], scale=10.0)
                nc.vector.tensor_reduce(at[:, ti:ti + 1], ts[:, :tw], X, ADD)

            if negw:
                mm_range(lhsT, neg[0], neg[1], neg_act, lead=lead)

            nl = rowp.tile([PBLK, 1], f32, name="nl", tag="nl")
            if negw:
                nc.vector.tensor_reduce(nl[:], at[:], X, ADD)
            else:
                nc.vector.memset(nl[:], 0.0)

            # --- positive columns: e = exp(-l) = exp(-10r + 10); NL enters the
            # ln pass as a per-row scale, so this pass has no NL dependency ---
            tp = scr.tile([PBLK, posmax], f32, name="tp", tag="tp")

            def pos_act(ps, tw, ti, tp=tp):
                off = ti * PS_W
                nc.scalar.activation(tp[:, off:off + tw], ps[:, :tw], Exp,
                                     bias=b10p[:], scale=-10.0)

            if not posw:
                continue  # degenerate single-label case; cout stays zero
            mm_range(lhsT, pos[0], pos[1], pos_act)

            # emitted after the pos exps so its NL dependency never bubbles ACT
            l1p = rowp.tile([PBLK, 1], f32, name="l1p", tag="l1p")
            nc.scalar.activation(l1p[:], nl[:], Ln, bias=1.0, scale=1.0)

            # --- sum_pos [l - ln(exp l + NL)] = -sum_pos ln(1 + NL*e) ---
            tz = scr.tile([PBLK, posmax], f32, name="tz", tag="tz")
            sz = rowp.tile([PBLK, 1], f32, name="sz", tag="sz")
            nc.scalar.activation(tz[:, :posw], tp[:, :posw], Ln,
                                 bias=1.0, scale=nl[:], accum_out=sz[:])

            # --- row combine: S = log1p(NL) - sz; out = S*w (w negated) ---
            s2 = rowp.tile([PBLK, 1], f32, name="s2", tag="s2")
            nc.vector.tensor_sub(s2[:], sz[:], l1p[:])
            nc.vector.tensor_mul(cout[:, rb:rb + 1], s2[:], wt[:, rb:rb + 1])
            if rb == RB - 2:
                nc.sync.dma_start(out=out_d[:, :RB - 1], in_=cout[:, :RB - 1])

        nc.sync.dma_start(out=out_d[:, RB - 1:], in_=cout[:, RB - 1:])

    _pin_act_table(nc)
    nc.compile()
    return nc


def _pin_act_table(nc, set_name="natural_log_exp_and_others"):
    """The default table chooser alternates exp-only/ln-only sets, paying a
    ~1.3us ACT_TABLE_LOAD on every Exp<->Ln transition. All our activation
    funcs live in one named set; offer the pass only that set (with a
    coverage check so a future func change falls back safely)."""
    from concourse.hw_specs import get_activation_tables
    from concourse import _compat  # noqa: F401
    import concourse.bacc as bacc_mod

    used = {
        inst.func
        for b in nc.main_func.blocks
        for inst in b.instructions
        if isinstance(inst, mybir.InstActivation)
    }
    tables = get_activation_tables(nc.m.arch)
    pinned = tables.get(set_name)
    if pinned is None or not used.issubset(pinned):
        return  # fall back to default behavior

    # act_func_set_id is the INDEX into act_info.json's act_func_sets, so the
    # list must keep its length/order; empty the other sets so the chooser
    # can only pick ours.
    masked = [(n, f if n == set_name else set()) for n, f in tables.items()]

    def patched():
        bacc_mod._bass_rust.insert_act_table_loads(nc, masked)

    nc.insert_act_table_loads = patched


def _prepare(feats, labels):
    """Host-side sharding prep: sort by label, build per-core inputs."""
    F = np.ascontiguousarray(np.asarray(feats, dtype=np.float32).reshape(N, D))
    lab = np.asarray(labels).reshape(N)
    perm = np.argsort(lab, kind="stable")
    Fs = F[perm]
    ys = lab[perm]
    cnt0 = int(np.searchsorted(ys, 1))
    cnt1 = N - cnt0
    k0 = math.ceil(cnt0 / 1024)

    ftb = np.ascontiguousarray(Fs.T.astype(ml_dtypes.bfloat16))  # [D, N]

    # global slot lists (indices into sorted order) + weights
    n0slots = NCORES * k0 * PBLK
    n1slots = NCORES * (RB - k0) * PBLK
    idx0 = np.full(n0slots, max(cnt0 - 1, 0), dtype=np.int64)
    idx0[:cnt0] = np.arange(cnt0)
    w0 = np.zeros(n0slots, dtype=np.float32)
    if cnt0 > 1:
        w0[:cnt0] = -1.0 / (cnt0 - 1)  # negated: device emits -S per row
    idx1 = np.full(n1slots, N - 1, dtype=np.int64)
    idx1[:cnt1] = cnt0 + np.arange(cnt1)
    w1 = np.zeros(n1slots, dtype=np.float32)
    if cnt1 > 1:
        w1[:cnt1] = -1.0 / (cnt1 - 1)

    idx0 = idx0.reshape(NCORES, k0 * PBLK)
    w0 = w0.reshape(NCORES, k0 * PBLK)
    idx1 = idx1.reshape(NCORES, (RB - k0) * PBLK)
    w1 = w1.reshape(NCORES, (RB - k0) * PBLK)

    in_maps = []
    for c in range(NCORES):
        rows = np.concatenate([idx0[c], idx1[c]])
        ftr = np.ascontiguousarray(Fs[rows].T.astype(ml_dtypes.bfloat16))
        wc = np.concatenate([w0[c], w1[c]]).reshape(RB, PBLK).T.copy()
        in_maps.append({"ftb": ftb, "ftr": ftr, "w": wc})
    return cnt0, in_maps


def _assemble(results):
    total = 0.0
    for r in results:
        total += float(r["out"].astype(np.float64).sum())
    return np.float32(-(TEMPERATURE / BASE_TEMPERATURE) * total / N)


_CACHE = {}


def kernel(feats, labels):
    cnt0, in_maps = _prepare(feats, labels)
    nc = _CACHE.get(cnt0)
    if nc is None:
        nc = _CACHE[cnt0] = _build(cnt0)
    res = run_bass_kernel_spmd(nc, in_maps, list(range(NCORES))).results
    return _assemble(res)
